# revision 1
# baseline (speedup 1.0000x reference)
"""APPNP-style GNN message passing on 8 Trainium2 NeuronCores.

Math (matches the PyG-default reference):
    h0 = (relu(x @ w1 + b1)) @ w2 + b2                       # MLP, [N, C]
    deg[v] = indegree(v) + 1 ; dinv = 1/sqrt(deg)
    repeat K times:
        h <- 0.9 * D^-1/2 (A + I) D^-1/2 h + 0.1 * h0
    out = log_softmax(h, axis=1)

Distribution (8 cores, v2):
  * Nodes degree-sorted and dealt into t tiles of 128 destinations per
    core.  Flat table row id of node (c, k, p) is (c*128 + p)*t + k.
  * Replicated DRAM table of g = dinv * h, fp16 rows PADDED to 128
    elements (256 B) so dma_gather's 256-byte element rule is met while
    matmuls consume the fp16 payload directly (cols 0:64).
  * Per hop, each core gathers source rows per (group of 2 dest tiles,
    32768-row index window) with dma_gather on 4 SWDGE queues (queue
    q's descriptors are generated by Q7 core pair (2q, 2q+1), so four
    queues run descriptor generation concurrently), then scatter-
    reduces with PE matmuls against one-hot selection matrices built
    by a single batched broadcast is_equal per group on DVE:
    psum[dest, f] += sum_pos S[pos, dest] * G[pos, f].
  * Epilogue per tile is one fused DVE scalar_tensor_tensor:
    g <- (0.9*dinv^2) * psum + (0.1*dinv) * h0   (next-hop table row)
    and on the last hop  h <- (0.9*dinv) * psum + 0.1*h0  -> log_softmax.
  * The updated g shard is AllGathered into every core's table.
"""

import sys

for _p in ("/opt/trn_rl_repo",):
    if _p not in sys.path:
        sys.path.insert(0, _p)

import numpy as np

import concourse.bacc as bacc
import concourse.mybir as mybir
import concourse.tile as tile

AF = mybir.ActivationFunctionType
ALU = mybir.AluOpType
DT = mybir.dt

N = 100000
E = 3200000
F_IN = 512
HID = 64
C = 64
CP = 128         # padded row width (fp16) -> 256 B
K = 10
ALPHA = 0.1

R = 8            # cores
P = 128          # partitions
WINDOW = 32768   # int16 index reach of dma_gather
GROUP = 2        # dest tiles sharing one grid buffer / gather set
NQ = 4           # SWDGE queues used round-robin for gathers


# --------------------------------------------------------------------------
# Host-side preprocessing
# --------------------------------------------------------------------------

def make_schedule(mcom, group=GROUP):
    """Common compile-time schedule from chunk counts mcom [t, nwin].

    Position layout: for each tile-group g, for each window q, for each
    tile k in g: a band of mcom[k, q]*128 positions.  One dma_gather per
    (g, q) covers that (contiguous) run.  Returns list of group dicts.
    """
    t, nwin = mcom.shape
    sched = []
    pos0 = 0
    for g0 in range(0, t, group):
        tiles = list(range(g0, min(g0 + group, t)))
        gathers = []
        tile_chunks = {k: [] for k in tiles}
        gslot = 0
        grp_pos0 = pos0
        for q in range(nwin):
            npos = int(mcom[tiles, q].sum()) * P
            if npos == 0:
                continue
            gathers.append((pos0, npos, gslot, q))
            for k in tiles:
                m = int(mcom[k, q])
                if m:
                    tile_chunks[k].append((gslot, m))
                    gslot += m
            pos0 += npos
        sched.append({
            "gathers": gathers,                  # (pos0, npos, slot0, q)
            "tiles": [(k, tile_chunks[k]) for k in tiles],
            "mg": gslot,
            "pos0": grp_pos0,
        })
    return sched, pos0                           # total positions


def preprocess(edge_index, n, r=R, p=P, group=GROUP):
    row = np.asarray(edge_index[0], dtype=np.int64)
    col = np.asarray(edge_index[1], dtype=np.int64)

    indeg = np.bincount(col, minlength=n)
    deg = indeg + 1                              # + self loop
    dinv = (1.0 / np.sqrt(deg.astype(np.float64))).astype(np.float32)

    block = r * p
    t = -(-n // block)
    nt = block * t
    shard = p * t
    nwin = -(-nt // WINDOW)

    order = np.argsort(-deg, kind="stable")
    sorted_nodes = np.concatenate([order, np.arange(n, nt)])
    q_of = np.empty(nt, dtype=np.int64)
    q_of[sorted_nodes] = np.arange(nt)

    # round-robin deal over (tile, core) so every tile gets an even
    # slice of the degree distribution -> uniform chunk counts
    k_of = q_of % t
    c_of = (q_of // t) % r
    p_of = q_of // (t * r)
    rowid = (c_of * p + p_of) * t + k_of         # node -> output row

    # k-major windowed table layout: window w covers k in [w*wk,(w+1)*wk);
    # within a window rows are core-major so each window is the contiguous
    # output region of one per-window AllGather chunk.  Balanced windows
    # (ceil(t/nwin) k's each, vs filling to the int16 limit) spread gather
    # descriptor generation evenly over the 4 SWDGE queue pairs.
    wk = -(-t // nwin)                           # k's per window
    assert r * p * wk <= WINDOW
    w_of = k_of // wk
    kb = w_of * wk
    kw_of = np.minimum(wk, t - kb)               # window k-extent per node
    w0 = np.zeros(nwin + 1, dtype=np.int64)
    for w in range(nwin):
        w0[w + 1] = w0[w] + r * p * min(wk, t - w * wk)
    rowid_tab = w0[w_of] + c_of * (p * kw_of) + (k_of - kb) * p + p_of

    # self loops are injected on-device via an identity matmul over the
    # local g shard, so only the real edges are gathered
    srcs = row
    dsts = col
    src_rid = rowid_tab[srcs]
    win = w_of[srcs]

    keys = (c_of[dsts] * t + k_of[dsts]) * nwin + win
    eorder = np.argsort(keys, kind="stable")
    e_key = keys[eorder]
    e_src = src_rid[eorder]
    e_win = win[eorder]
    e_dst_p = p_of[dsts][eorder]

    counts = np.bincount(e_key, minlength=r * t * nwin).reshape(r, t, nwin)
    mcom = (-(-counts // p)).max(axis=0)                     # [t, nwin]

    sched, tot_pos = make_schedule(mcom, group)
    tot_slot = tot_pos // p

    # band start positions under the common schedule
    band_start = np.zeros((t, nwin), dtype=np.int64)
    pos0 = 0
    for g0 in range(0, t, group):
        tiles = range(g0, min(g0 + group, t))
        for q in range(nwin):
            for k in tiles:
                band_start[k, q] = pos0
                pos0 += int(mcom[k, q]) * p
    assert pos0 == tot_pos

    starts = np.concatenate([[0], np.cumsum(counts.reshape(-1))])
    erank = np.arange(e_key.shape[0]) - starts[e_key]
    e_c = e_key // (t * nwin)
    e_k = (e_key // nwin) % t
    e_pos = band_start[e_k, e_win] + erank

    idx16 = np.zeros((r, tot_pos), dtype=np.int16)
    idx16[e_c, e_pos] = (e_src - w0[e_win]).astype(np.int16)

    # selection column ids (-1 at padding) for the batched is_equal
    dcol = np.full((r, p, tot_slot), -1.0, dtype=np.float16)
    dcol[e_c, e_pos % p, e_pos // p] = e_dst_p.astype(np.float16)

    # wrap indices: j -> [j%16, j//16], replicated over the 8 Q7 groups
    w16 = idx16.reshape(r, tot_pos // 16, 16).transpose(0, 2, 1)
    idx16w = np.ascontiguousarray(np.tile(w16, (1, 8, 1)))   # [r, 128, tp/16]

    inv = np.empty(nt, dtype=np.int64)
    inv[rowid] = np.arange(nt)
    shard_nodes = inv.reshape(r, shard)

    dinv_pad = np.zeros(nt, dtype=np.float32)
    dinv_pad[:n] = dinv
    dinv_pk = dinv_pad[shard_nodes].reshape(r, p, t)

    return {
        "t": t, "nt": nt, "shard": shard, "nwin": nwin,
        "mcom": mcom, "sched": sched, "tot_pos": tot_pos,
        "tot_slot": tot_slot,
        "idx16w": idx16w, "dcol": np.ascontiguousarray(dcol),
        "rowid": rowid, "shard_nodes": shard_nodes,
        "dinv_pk": np.ascontiguousarray(dinv_pk),
    }


# --------------------------------------------------------------------------
# Bass program
# --------------------------------------------------------------------------

def build_program(t, nt, nwin, sched, tot_pos, tot_slot,
                  f_in=F_IN, k_hops=K, alpha=ALPHA, r=R, nqueues=NQ):
    shard = P * t
    fc = f_in // P
    assert f_in % P == 0

    nc = bacc.Bacc("TRN2", target_bir_lowering=False, debug=False,
                   num_devices=r, num_swdge_queues=nqueues)

    xT = nc.dram_tensor("xT", [f_in, shard], DT.float16, kind="ExternalInput")
    w1r = nc.dram_tensor("w1r", [P, fc, HID], DT.float16, kind="ExternalInput")
    b1c = nc.dram_tensor("b1c", [HID, 1], DT.float32, kind="ExternalInput")
    w2m = nc.dram_tensor("w2m", [HID, C], DT.float16, kind="ExternalInput")
    b2r = nc.dram_tensor("b2r", [P, C], DT.float32, kind="ExternalInput")
    dinv_h = nc.dram_tensor("dinv", [P, t], DT.float32, kind="ExternalInput")
    dinv09_h = nc.dram_tensor("dinv09", [P, t], DT.float32,
                              kind="ExternalInput")
    scale2_h = nc.dram_tensor("scale2", [P, t], DT.float32,
                              kind="ExternalInput")
    adinv_h = nc.dram_tensor("adinv", [P, t], DT.float32,
                             kind="ExternalInput")
    idx_h = nc.dram_tensor("idx16", [P, tot_pos // 16], DT.int16,
                           kind="ExternalInput")
    dcol_h = nc.dram_tensor("dcol", [P, tot_slot], DT.float16,
                            kind="ExternalInput")
    iota_h = nc.dram_tensor("iota", [P, P], DT.float16, kind="ExternalInput")
    ident_h = nc.dram_tensor("ident", [P, P], DT.float16,
                             kind="ExternalInput")
    out_h = nc.dram_tensor("out", [P, t * C], DT.float32,
                           kind="ExternalOutput")

    groups = [list(range(r))]
    mg_max = max(g["mg"] for g in sched)

    with tile.TileContext(nc) as tc:
        with (
            tc.tile_pool(name="const", bufs=1) as cpool,
            tc.tile_pool(name="xin", bufs=3) as xpool,
            tc.tile_pool(name="mlp", bufs=3) as mpool,
            tc.tile_pool(name="grid", bufs=3) as gpool,
            tc.tile_pool(name="idxp", bufs=6) as ipool,
            tc.tile_pool(name="sel", bufs=2) as spool,
            tc.tile_pool(name="small", bufs=6) as apool,
            tc.tile_pool(name="cols", bufs=6) as colpool,
            tc.tile_pool(name="ps", bufs=2, space="PSUM") as pspool,
            tc.tile_pool(name="psb", bufs=4, space="PSUM") as psbpool,
            tc.tile_pool(name="dram", bufs=1, space="DRAM") as dpool,
        ):
            bounces = [dpool.tile([t * P, CP], DT.float16, name=f"bounce{i}")
                       for i in range(2)]
            tables = [dpool.tile([r * P * t, CP], DT.float16,
                                 name=f"table{i}")
                      for i in range(k_hops)]
            # window bounds: window w = k in [w*wk, (w+1)*wk), contiguous
            # core-major table region [w0[w], w0[w+1]); balanced across the
            # nwin windows to even out SWDGE queue load (must match
            # preprocess)
            wk = -(-t // nwin)
            kws = [min(wk, t - w * wk) for w in range(nwin)]
            w0 = [0]
            for w in range(nwin):
                w0.append(w0[-1] + r * P * kws[w])

            w1_s = cpool.tile([P, fc, HID], DT.float16)
            b1_s = cpool.tile([HID, 1], DT.float32)
            w2_s = cpool.tile([HID, C], DT.float16)
            b2_s = cpool.tile([P, C], DT.float32)
            dinv_s = cpool.tile([P, t], DT.float32)
            dinv09_s = cpool.tile([P, t], DT.float32)
            scale2_s = cpool.tile([P, t], DT.float32)
            adinv_s = cpool.tile([P, t], DT.float32)
            dcol_s = cpool.tile([P, tot_slot], DT.float16)
            iota_s = cpool.tile([P, P], DT.float16)
            ident_s = cpool.tile([P, P], DT.float16)
            h0s_buf = cpool.tile([P, t * C], DT.float16)   # 0.1 * h0
            h0g_buf = cpool.tile([P, t * C], DT.float16)   # 0.1 * dinv * h0
            g_buf = cpool.tile([P, t * CP], DT.float16)    # dinv*h, 256B rows

            nc.sync.dma_start(out=w1_s[:], in_=w1r.ap())
            nc.sync.dma_start(out=b1_s[:], in_=b1c.ap())
            nc.sync.dma_start(out=w2_s[:], in_=w2m.ap())
            nc.sync.dma_start(out=b2_s[:], in_=b2r.ap())
            nc.sync.dma_start(out=dinv_s[:], in_=dinv_h.ap())
            nc.sync.dma_start(out=dinv09_s[:], in_=dinv09_h.ap())
            nc.sync.dma_start(out=scale2_s[:], in_=scale2_h.ap())
            nc.sync.dma_start(out=adinv_s[:], in_=adinv_h.ap())
            nc.sync.dma_start(out=dcol_s[:], in_=dcol_h.ap())
            nc.sync.dma_start(out=iota_s[:], in_=iota_h.ap())
            nc.sync.dma_start(out=ident_s[:], in_=ident_h.ap())

            # zero the padded halves of g rows once; only cols 0:C are
            # rewritten afterwards
            nc.vector.memset(g_buf[:], 0.0)

            xT_r = xT.ap().rearrange("(c p) n -> p c n", p=P)
            tables_flat = [tb[:] for tb in tables]
            g3 = g_buf[:].rearrange("p (k f) -> p k f", f=CP)
            h03 = h0s_buf[:].rearrange("p (k f) -> p k f", f=C)
            h0g3 = h0g_buf[:].rearrange("p (k f) -> p k f", f=C)

            # ---------------- MLP ----------------------------------------
            # the initial table AllGather is chunked per window and fires
            # as soon as the MLP finishes that window's k range, overlapping
            # the remaining MLP tiles
            b0v = bounces[0][:].rearrange("(k p) f -> p k f", p=P)
            next_w0 = 0
            for kt in range(t):
                xt = xpool.tile([P, fc, P], DT.float16, tag="xt")
                nc.sync.dma_start(out=xt[:], in_=xT_r[:, :, kt * P:(kt + 1) * P])
                ps1 = pspool.tile([HID, P], DT.float32, tag="ps1")
                for ci in range(fc):
                    nc.tensor.matmul(ps1[:], lhsT=w1_s[:, ci, :],
                                     rhs=xt[:, ci, :],
                                     start=(ci == 0), stop=(ci == fc - 1))
                h1 = mpool.tile([HID, P], DT.float16, tag="h1")
                nc.scalar.activation(h1[:], ps1[:], AF.Relu, bias=b1_s[:, 0:1])
                ps3 = pspool.tile([P, C], DT.float32, tag="ps3")
                nc.tensor.matmul(ps3[:], lhsT=h1[:], rhs=w2_s[:],
                                 start=True, stop=True)
                hb = mpool.tile([P, C], DT.float32, tag="hb")
                nc.vector.tensor_add(out=hb[:], in0=ps3[:], in1=b2_s[:])
                nc.scalar.mul(h03[:, kt, :], hb[:], alpha)
                nc.vector.tensor_scalar_mul(h0g3[:, kt, :], hb[:],
                                            adinv_s[:, kt:kt + 1])
                nc.vector.tensor_scalar_mul(g3[:, kt, 0:C], hb[:],
                                            dinv_s[:, kt:kt + 1])
                while (next_w0 < nwin
                       and kt + 1 >= wk * next_w0 + kws[next_w0]):
                    kb0 = wk * next_w0
                    ke0 = kb0 + kws[next_w0]
                    nc.scalar.dma_start(out=b0v[:, kb0:ke0, :],
                                        in_=g3[:, kb0:ke0, :])
                    nc.gpsimd.collective_compute(
                        "AllGather", ALU.bypass, replica_groups=groups,
                        ins=[bounces[0][:][kb0 * P:ke0 * P, :].opt()],
                        outs=[tables[0][:][w0[next_w0]:
                                           w0[next_w0 + 1], :].opt()])
                    next_w0 += 1

            # ---------------- K propagation hops -------------------------
            gq = 0
            for hop in range(k_hops):
                last = hop == k_hops - 1
                table_flat = tables_flat[hop]
                next_w = 0
                for gi, grp in enumerate(sched):
                    mg = grp["mg"]
                    gbase = grp["pos0"] // P          # global slot base
                    npos_grp = mg * P
                    i0 = grp["pos0"] // 16
                    it = ipool.tile([P, mg * P // 16], DT.int16, tag="idx")
                    nc.sync.dma_start(
                        out=it[:],
                        in_=idx_h.ap()[:, i0:i0 + npos_grp // 16])
                    grid = gpool.tile([P, mg_max, CP], DT.float16, tag="grid")
                    for qi, (gpos0, npos, slot0, q) in enumerate(grp["gathers"]):
                        o0 = (gpos0 - grp["pos0"]) // 16
                        nc.gpsimd.dma_gather(
                            out_ap=grid[:, slot0:slot0 + npos // P, :],
                            in_ap=table_flat[w0[q]:w0[q + 1], :],
                            idxs_ap=it[:, o0:o0 + npos // 16],
                            num_idxs=npos,
                            num_idxs_reg=npos,
                            elem_size=CP,
                            single_packet=False,
                            queue_num=(gi + qi) % nqueues,
                        )
                        gq += 1
                    # one batched one-hot build for the whole group
                    sel = spool.tile([P, mg_max, P], DT.float16, tag="S")
                    nc.vector.tensor_tensor(
                        out=sel[:, 0:mg, :],
                        in0=iota_s[:].rearrange("p (o j) -> p o j", o=1)
                                     .broadcast_to([P, mg, P]),
                        in1=dcol_s[:, gbase:gbase + mg]
                                  .rearrange("p (m o) -> p m o", o=1)
                                  .broadcast_to([P, mg, P]),
                        op=ALU.is_equal)
                    for (kt, chunks) in grp["tiles"]:
                        psA = psbpool.tile([P, C], DT.float32, tag="agg")
                        nchunks = sum(m for _, m in chunks)
                        # self loop: previous hop's g row for this tile is
                        # still resident in g_buf (epilogue rewrites it only
                        # after these matmuls)
                        nc.tensor.matmul(
                            psA[:], lhsT=ident_s[:], rhs=g3[:, kt, 0:C],
                            start=True, stop=(nchunks == 0))
                        done = 0
                        for (slot0, m) in chunks:
                            for s in range(slot0, slot0 + m):
                                nc.tensor.matmul(
                                    psA[:], lhsT=sel[:, s, :],
                                    rhs=grid[:, s, 0:C],
                                    start=False,
                                    stop=(done == nchunks - 1))
                                done += 1
                        if not last:
                            # g <- scale2 * agg + 0.1*dinv*h0  (fp16)
                            nc.vector.scalar_tensor_tensor(
                                out=g3[:, kt, 0:C], in0=psA[:],
                                scalar=scale2_s[:, kt:kt + 1],
                                in1=h0g3[:, kt, :],
                                op0=ALU.mult, op1=ALU.add)
                        else:
                            hn = apool.tile([P, C], DT.float32, tag="hn")
                            nc.vector.scalar_tensor_tensor(
                                out=hn[:], in0=psA[:],
                                scalar=dinv09_s[:, kt:kt + 1],
                                in1=h03[:, kt, :],
                                op0=ALU.mult, op1=ALU.add)
                            mx = colpool.tile([P, 1], DT.float32, tag="mx")
                            nc.vector.reduce_max(mx[:], hn[:],
                                                 axis=mybir.AxisListType.X,
                                                 negate=True)       # -max
                            ex = apool.tile([P, C], DT.float32, tag="ex")
                            ssum = colpool.tile([P, 1], DT.float32, tag="ssum")
                            nc.scalar.activation(ex[:], hn[:], AF.Exp,
                                                 bias=mx[:, 0:1],
                                                 accum_out=ssum[:, 0:1])
                            lg = colpool.tile([P, 1], DT.float32, tag="lg")
                            nc.scalar.activation(lg[:], ssum[:], AF.Ln)
                            mpl = colpool.tile([P, 1], DT.float32, tag="mpl")
                            nc.vector.tensor_tensor(out=mpl[:], in0=lg[:],
                                                    in1=mx[:],
                                                    op=ALU.subtract)
                            res = apool.tile([P, C], DT.float32, tag="res")
                            nc.vector.tensor_scalar(
                                out=res[:], in0=hn[:],
                                scalar1=mpl[:, 0:1], scalar2=None,
                                op0=ALU.subtract)
                            ksl = slice(kt * C, (kt + 1) * C)
                            nc.sync.dma_start(out=out_h.ap()[:, ksl],
                                              in_=res[:])
                    if not last:
                        # stream this group's refreshed g rows out to the
                        # bounce buffer so each window's AllGather can fire
                        # the moment its last group's epilogue lands;
                        # ACT-issued so the Sync queue (idx loads) never
                        # stalls behind it
                        bnc = bounces[(hop + 1) % 2]
                        b3 = bnc[:].rearrange("(k p) f -> p k f", p=P)
                        k0 = grp["tiles"][0][0]
                        k1 = grp["tiles"][-1][0] + 1
                        nc.scalar.dma_start(
                            out=b3[:, k0:k1, :],
                            in_=g3[:, k0:k1, :])
                        while next_w < nwin and k1 >= wk * next_w + kws[next_w]:
                            nc.gpsimd.collective_compute(
                                "AllGather", ALU.bypass,
                                replica_groups=groups,
                                ins=[bnc[:][wk * next_w * P:
                                            (wk * next_w + kws[next_w]) * P,
                                            :].opt()],
                                outs=[tables[hop + 1][:][
                                    w0[next_w]:w0[next_w + 1], :].opt()])
                            next_w += 1

    nc.compile()
    return nc


# --------------------------------------------------------------------------
# in_maps assembly
# --------------------------------------------------------------------------

def make_in_maps(x, w1, b1, w2, b2, pre, f_in=F_IN, r=R):
    n = x.shape[0]
    t, nt = pre["t"], pre["nt"]
    fc = f_in // P

    xp = np.zeros((nt, f_in), dtype=np.float16)
    xp[:n] = np.asarray(x, dtype=np.float16)
    w1r = np.ascontiguousarray(
        np.asarray(w1, np.float16).reshape(fc, P, HID).transpose(1, 0, 2))
    b1c = np.ascontiguousarray(np.asarray(b1, np.float32).reshape(HID, 1))
    w2m = np.ascontiguousarray(np.asarray(w2, np.float16))
    b2r = np.ascontiguousarray(
        np.tile(np.asarray(b2, np.float32).reshape(1, C), (P, 1)))
    iota = np.ascontiguousarray(
        np.tile(np.arange(P, dtype=np.float16).reshape(1, P), (P, 1)))
    ident = np.eye(P, dtype=np.float16)

    in_maps = []
    for c in range(r):
        nodes = pre["shard_nodes"][c].reshape(P, t).T.reshape(-1)  # k-major
        xT_c = np.ascontiguousarray(xp[nodes].T)
        dpk = pre["dinv_pk"][c]
        in_maps.append({
            "xT": xT_c,
            "w1r": w1r, "b1c": b1c, "w2m": w2m, "b2r": b2r,
            "dinv": np.ascontiguousarray(dpk),
            "dinv09": np.ascontiguousarray(0.9 * dpk),
            "scale2": np.ascontiguousarray(0.9 * dpk * dpk),
            "adinv": np.ascontiguousarray(ALPHA * dpk),
            "idx16": pre["idx16w"][c],
            "dcol": pre["dcol"][c],
            "iota": iota,
            "ident": ident,
        })
    return in_maps


_CACHE = {}


def kernel(x, edge_index, w1, b1, w2, b2):
    from concourse.bass_utils import run_bass_kernel_spmd

    x = np.asarray(x)
    n = x.shape[0]
    pre = preprocess(np.asarray(edge_index), n)
    key = (pre["t"], pre["tot_pos"], tuple(pre["mcom"].reshape(-1)))
    if key not in _CACHE:
        _CACHE[key] = build_program(pre["t"], pre["nt"], pre["nwin"],
                                    pre["sched"], pre["tot_pos"],
                                    pre["tot_slot"])
    nc = _CACHE[key]

    in_maps = make_in_maps(x, w1, b1, w2, b2, pre)
    res = run_bass_kernel_spmd(nc, in_maps, core_ids=list(range(R)))
    outs = np.stack([res.results[c]["out"] for c in range(R)])
    flat = outs.reshape(R * pre["shard"], C)
    return np.ascontiguousarray(flat[pre["rowid"][:n]]).astype(np.float32)



# revision 16
# speedup vs baseline: 1.0256x; 1.0256x over previous
"""APPNP-style GNN message passing on 8 Trainium2 NeuronCores.

Math (matches the PyG-default reference):
    h0 = (relu(x @ w1 + b1)) @ w2 + b2                       # MLP, [N, C]
    deg[v] = indegree(v) + 1 ; dinv = 1/sqrt(deg)
    repeat K times:
        h <- 0.9 * D^-1/2 (A + I) D^-1/2 h + 0.1 * h0
    out = log_softmax(h, axis=1)

Distribution (8 cores, v2):
  * Nodes degree-sorted and dealt into t tiles of 128 destinations per
    core.  Flat table row id of node (c, k, p) is (c*128 + p)*t + k.
  * Replicated DRAM table of g = dinv * h, fp16 rows PADDED to 128
    elements (256 B) so dma_gather's 256-byte element rule is met while
    matmuls consume the fp16 payload directly (cols 0:64).
  * Per hop, each core gathers source rows per (group of 2 dest tiles,
    32768-row index window) with dma_gather on 4 SWDGE queues (queue
    q's descriptors are generated by Q7 core pair (2q, 2q+1), so four
    queues run descriptor generation concurrently), then scatter-
    reduces with PE matmuls against one-hot selection matrices built
    by a single batched broadcast is_equal per group on DVE:
    psum[dest, f] += sum_pos S[pos, dest] * G[pos, f].
  * Epilogue per tile is one fused DVE scalar_tensor_tensor:
    g <- (0.9*dinv^2) * psum + (0.1*dinv) * h0   (next-hop table row)
    and on the last hop  h <- (0.9*dinv) * psum + 0.1*h0  -> log_softmax.
  * The updated g shard is AllGathered into every core's table.
"""

import sys

for _p in ("/opt/trn_rl_repo",):
    if _p not in sys.path:
        sys.path.insert(0, _p)

import numpy as np

import concourse.bacc as bacc
import concourse.mybir as mybir
import concourse.tile as tile

AF = mybir.ActivationFunctionType
ALU = mybir.AluOpType
DT = mybir.dt

N = 100000
E = 3200000
F_IN = 512
HID = 64
C = 64
CP = 128         # padded row width (fp16) -> 256 B
K = 10
ALPHA = 0.1

R = 8            # cores
P = 128          # partitions
WINDOW = 32768   # int16 index reach of dma_gather
GROUP = 2        # dest tiles sharing one grid buffer / gather set
NQ = 4           # SWDGE queues used round-robin for gathers


# --------------------------------------------------------------------------
# Host-side preprocessing
# --------------------------------------------------------------------------

def make_schedule(mcom, group=GROUP):
    """Common compile-time schedule from chunk counts mcom [t, nwin].

    Position layout: for each tile-group g, for each window q, for each
    tile k in g: a band of mcom[k, q]*128 positions.  One dma_gather per
    (g, q) covers that (contiguous) run.  Returns list of group dicts.
    """
    t, nwin = mcom.shape
    sched = []
    pos0 = 0
    for g0 in range(0, t, group):
        tiles = list(range(g0, min(g0 + group, t)))
        gathers = []
        tile_chunks = {k: [] for k in tiles}
        gslot = 0
        grp_pos0 = pos0
        for q in range(nwin):
            npos = int(mcom[tiles, q].sum()) * P
            if npos == 0:
                continue
            gathers.append((pos0, npos, gslot, q))
            for k in tiles:
                m = int(mcom[k, q])
                if m:
                    tile_chunks[k].append((gslot, m))
                    gslot += m
            pos0 += npos
        sched.append({
            "gathers": gathers,                  # (pos0, npos, slot0, q)
            "tiles": [(k, tile_chunks[k]) for k in tiles],
            "mg": gslot,
            "pos0": grp_pos0,
        })
    return sched, pos0                           # total positions


def preprocess(edge_index, n, r=R, p=P, group=GROUP):
    row = np.asarray(edge_index[0], dtype=np.int64)
    col = np.asarray(edge_index[1], dtype=np.int64)

    indeg = np.bincount(col, minlength=n)
    deg = indeg + 1                              # + self loop
    dinv = (1.0 / np.sqrt(deg.astype(np.float64))).astype(np.float32)

    block = r * p
    t = -(-n // block)
    nt = block * t
    shard = p * t
    nwin = -(-nt // WINDOW)

    order = np.argsort(-deg, kind="stable")
    sorted_nodes = np.concatenate([order, np.arange(n, nt)])
    q_of = np.empty(nt, dtype=np.int64)
    q_of[sorted_nodes] = np.arange(nt)

    # round-robin deal over (tile, core) so every tile gets an even
    # slice of the degree distribution -> uniform chunk counts
    k_of = q_of % t
    c_of = (q_of // t) % r
    p_of = q_of // (t * r)
    rowid = (c_of * p + p_of) * t + k_of         # node -> output row

    # k-major windowed table layout: window w covers k in [w*wk,(w+1)*wk);
    # within a window rows are core-major so each window is the contiguous
    # output region of one per-window AllGather chunk.  Balanced windows
    # (ceil(t/nwin) k's each, vs filling to the int16 limit) spread gather
    # descriptor generation evenly over the 4 SWDGE queue pairs.
    wk = -(-t // nwin)                           # k's per window
    assert r * p * wk <= WINDOW
    w_of = k_of // wk
    kb = w_of * wk
    kw_of = np.minimum(wk, t - kb)               # window k-extent per node
    w0 = np.zeros(nwin + 1, dtype=np.int64)
    for w in range(nwin):
        w0[w + 1] = w0[w] + r * p * min(wk, t - w * wk)
    rowid_tab = w0[w_of] + c_of * (p * kw_of) + (k_of - kb) * p + p_of

    # self loops are injected on-device via an identity matmul over the
    # local g shard, so only the real edges are gathered
    srcs = row
    dsts = col
    src_rid = rowid_tab[srcs]
    win = w_of[srcs]

    keys = (c_of[dsts] * t + k_of[dsts]) * nwin + win
    eorder = np.argsort(keys, kind="stable")
    e_key = keys[eorder]
    e_src = src_rid[eorder]
    e_win = win[eorder]
    e_dst_p = p_of[dsts][eorder]

    counts = np.bincount(e_key, minlength=r * t * nwin).reshape(r, t, nwin)
    mcom = (-(-counts // p)).max(axis=0)                     # [t, nwin]

    sched, tot_pos = make_schedule(mcom, group)
    tot_slot = tot_pos // p

    # band start positions under the common schedule
    band_start = np.zeros((t, nwin), dtype=np.int64)
    pos0 = 0
    for g0 in range(0, t, group):
        tiles = range(g0, min(g0 + group, t))
        for q in range(nwin):
            for k in tiles:
                band_start[k, q] = pos0
                pos0 += int(mcom[k, q]) * p
    assert pos0 == tot_pos

    starts = np.concatenate([[0], np.cumsum(counts.reshape(-1))])
    erank = np.arange(e_key.shape[0]) - starts[e_key]
    e_c = e_key // (t * nwin)
    e_k = (e_key // nwin) % t
    e_pos = band_start[e_k, e_win] + erank

    # padding positions point at a guaranteed-zero table row of the right
    # window (a pad node: dinv=0 keeps its g row zero every hop), except
    # trailing pad runs at the end of each gather band, which use -1 so the
    # SWDGE ucode trims them before descriptor generation (saves ~half the
    # padding DMA traffic)
    zrow = np.empty(nwin, dtype=np.int64)
    for w in range(nwin):
        cand = np.nonzero(w_of[np.arange(n, nt)] == w)[0]
        assert cand.size, f"no pad node in window {w}"
        zrow[w] = rowid_tab[n + cand[0]] - w0[w]
    winpos = np.empty(tot_pos, dtype=np.int64)
    for g0 in range(0, t, group):
        tiles = range(g0, min(g0 + group, t))
        for q in range(nwin):
            for k in tiles:
                winpos[band_start[k, q]:band_start[k, q]
                       + int(mcom[k, q]) * p] = q
    idx16 = np.tile(zrow[winpos].astype(np.int16), (r, 1))
    idx16[e_c, e_pos] = (e_src - w0[e_win]).astype(np.int16)

    # selection column ids (-1 at padding) for the batched is_equal
    dcol = np.full((r, p, tot_slot), -1.0, dtype=np.float16)
    dcol[e_c, e_pos % p, e_pos // p] = e_dst_p.astype(np.float16)

    # wrap indices: j -> [j%16, j//16], replicated over the 8 Q7 groups
    w16 = idx16.reshape(r, tot_pos // 16, 16).transpose(0, 2, 1)
    idx16w = np.ascontiguousarray(np.tile(w16, (1, 8, 1)))   # [r, 128, tp/16]

    inv = np.empty(nt, dtype=np.int64)
    inv[rowid] = np.arange(nt)
    shard_nodes = inv.reshape(r, shard)

    dinv_pad = np.zeros(nt, dtype=np.float32)
    dinv_pad[:n] = dinv
    dinv_pk = dinv_pad[shard_nodes].reshape(r, p, t)

    return {
        "t": t, "nt": nt, "shard": shard, "nwin": nwin,
        "mcom": mcom, "sched": sched, "tot_pos": tot_pos,
        "tot_slot": tot_slot,
        "idx16w": idx16w, "dcol": np.ascontiguousarray(dcol),
        "rowid": rowid, "shard_nodes": shard_nodes,
        "dinv_pk": np.ascontiguousarray(dinv_pk),
    }


# --------------------------------------------------------------------------
# Bass program
# --------------------------------------------------------------------------

def build_program(t, nt, nwin, sched, tot_pos, tot_slot,
                  f_in=F_IN, k_hops=K, alpha=ALPHA, r=R, nqueues=NQ):
    shard = P * t
    fc = f_in // P
    assert f_in % P == 0

    nc = bacc.Bacc("TRN2", target_bir_lowering=False, debug=False,
                   num_devices=r, num_swdge_queues=nqueues)

    xT = nc.dram_tensor("xT", [f_in, shard], DT.float16, kind="ExternalInput")
    w1r = nc.dram_tensor("w1r", [P, fc, HID], DT.float16, kind="ExternalInput")
    b1c = nc.dram_tensor("b1c", [HID, 1], DT.float32, kind="ExternalInput")
    w2m = nc.dram_tensor("w2m", [HID, C], DT.float16, kind="ExternalInput")
    b2r = nc.dram_tensor("b2r", [P, C], DT.float32, kind="ExternalInput")
    dinv_h = nc.dram_tensor("dinv", [P, t], DT.float32, kind="ExternalInput")
    dinv09_h = nc.dram_tensor("dinv09", [P, t], DT.float32,
                              kind="ExternalInput")
    scale2_h = nc.dram_tensor("scale2", [P, t], DT.float32,
                              kind="ExternalInput")
    adinv_h = nc.dram_tensor("adinv", [P, t], DT.float32,
                             kind="ExternalInput")
    idx_h = nc.dram_tensor("idx16", [P, tot_pos // 16], DT.int16,
                           kind="ExternalInput")
    dcol_h = nc.dram_tensor("dcol", [P, tot_slot], DT.float16,
                            kind="ExternalInput")
    iota_h = nc.dram_tensor("iota", [P, P], DT.float16, kind="ExternalInput")
    ident_h = nc.dram_tensor("ident", [P, P], DT.float16,
                             kind="ExternalInput")
    out_h = nc.dram_tensor("out", [P, t * C], DT.float32,
                           kind="ExternalOutput")

    groups = [list(range(r))]
    mg_max = max(g["mg"] for g in sched)

    with tile.TileContext(nc) as tc:
        with (
            tc.tile_pool(name="const", bufs=1) as cpool,
            tc.tile_pool(name="xin", bufs=3) as xpool,
            tc.tile_pool(name="mlp", bufs=3) as mpool,
            tc.tile_pool(name="grid", bufs=3) as gpool,
            tc.tile_pool(name="idxp", bufs=6) as ipool,
            tc.tile_pool(name="sel", bufs=2) as spool,
            tc.tile_pool(name="small", bufs=6) as apool,
            tc.tile_pool(name="cols", bufs=6) as colpool,
            tc.tile_pool(name="ps", bufs=2, space="PSUM") as pspool,
            tc.tile_pool(name="psb", bufs=4, space="PSUM") as psbpool,
            tc.tile_pool(name="dram", bufs=1, space="DRAM") as dpool,
        ):
            bounces = [dpool.tile([t * P, CP], DT.float16, name=f"bounce{i}")
                       for i in range(2)]
            # window bounds: window w = k in [w*wk, (w+1)*wk), contiguous
            # core-major table region [w0[w], w0[w+1]); balanced across the
            # nwin windows to even out SWDGE queue load (must match
            # preprocess)
            wk = -(-t // nwin)
            kws = [min(wk, t - w * wk) for w in range(nwin)]
            w0 = [0]
            for w in range(nwin):
                w0.append(w0[-1] + r * P * kws[w])
            # One tensor per (hop, window). (addr_space="Shared" crashes the
            # axon PJRT runtime with an INTERNAL error — keep Local.)
            tables = [[dpool.tile([r * P * kws[w], CP], DT.float16,
                                  name=f"table{i}w{w}")
                       for w in range(nwin)]
                      for i in range(k_hops)]

            w1_s = cpool.tile([P, fc, HID], DT.float16)
            b1_s = cpool.tile([HID, 1], DT.float32)
            w2_s = cpool.tile([HID, C], DT.float16)
            b2_s = cpool.tile([P, C], DT.float32)
            dinv_s = cpool.tile([P, t], DT.float32)
            dinv09_s = cpool.tile([P, t], DT.float32)
            scale2_s = cpool.tile([P, t], DT.float32)
            adinv_s = cpool.tile([P, t], DT.float32)
            dcol_s = cpool.tile([P, tot_slot], DT.float16)
            iota_s = cpool.tile([P, P], DT.float16)
            ident_s = cpool.tile([P, P], DT.float16)
            h0s_buf = cpool.tile([P, t * C], DT.float16)   # 0.1 * h0
            h0g_buf = cpool.tile([P, t * C], DT.float16)   # 0.1 * dinv * h0
            g_buf = cpool.tile([P, t * CP], DT.float16)    # dinv*h, 256B rows

            nc.sync.dma_start(out=w1_s[:], in_=w1r.ap())
            nc.sync.dma_start(out=b1_s[:], in_=b1c.ap())
            nc.sync.dma_start(out=w2_s[:], in_=w2m.ap())
            nc.sync.dma_start(out=b2_s[:], in_=b2r.ap())
            nc.sync.dma_start(out=dinv_s[:], in_=dinv_h.ap())
            nc.sync.dma_start(out=dinv09_s[:], in_=dinv09_h.ap())
            nc.sync.dma_start(out=scale2_s[:], in_=scale2_h.ap())
            nc.sync.dma_start(out=adinv_s[:], in_=adinv_h.ap())
            nc.sync.dma_start(out=dcol_s[:], in_=dcol_h.ap())
            nc.sync.dma_start(out=iota_s[:], in_=iota_h.ap())
            nc.sync.dma_start(out=ident_s[:], in_=ident_h.ap())

            # zero the padded halves of g rows once; only cols 0:C are
            # rewritten afterwards
            nc.vector.memset(g_buf[:], 0.0)

            # zero all grid buffers once: positions skipped by -1 pad
            # indices keep stale buffer contents, which must be finite so
            # the zero sel columns actually cancel them (0*NaN == NaN)
            for _gz in range(3):
                gz = gpool.tile([P, mg_max, CP], DT.float16, tag="grid")
                nc.vector.memset(gz[:], 0.0)

            xT_r = xT.ap().rearrange("(c p) n -> p c n", p=P)
            tables_flat = [[tw[:] for tw in tb] for tb in tables]
            g3 = g_buf[:].rearrange("p (k f) -> p k f", f=CP)
            h03 = h0s_buf[:].rearrange("p (k f) -> p k f", f=C)
            h0g3 = h0g_buf[:].rearrange("p (k f) -> p k f", f=C)

            # ---------------- MLP ----------------------------------------
            # the initial table AllGather is chunked per window and fires
            # as soon as the MLP finishes that window's k range, overlapping
            # the remaining MLP tiles
            b0v = bounces[0][:].rearrange("(k p) f -> p k f", p=P)
            next_w0 = 0
            for kt in range(t):
                xt = xpool.tile([P, fc, P], DT.float16, tag="xt")
                nc.sync.dma_start(out=xt[:], in_=xT_r[:, :, kt * P:(kt + 1) * P])
                ps1 = pspool.tile([HID, P], DT.float32, tag="ps1")
                for ci in range(fc):
                    nc.tensor.matmul(ps1[:], lhsT=w1_s[:, ci, :],
                                     rhs=xt[:, ci, :],
                                     start=(ci == 0), stop=(ci == fc - 1))
                h1 = mpool.tile([HID, P], DT.float16, tag="h1")
                nc.scalar.activation(h1[:], ps1[:], AF.Relu, bias=b1_s[:, 0:1])
                ps3 = pspool.tile([P, C], DT.float32, tag="ps3")
                nc.tensor.matmul(ps3[:], lhsT=h1[:], rhs=w2_s[:],
                                 start=True, stop=True)
                hb = mpool.tile([P, C], DT.float32, tag="hb")
                nc.vector.tensor_add(out=hb[:], in0=ps3[:], in1=b2_s[:])
                nc.scalar.mul(h03[:, kt, :], hb[:], alpha)
                nc.vector.tensor_scalar_mul(h0g3[:, kt, :], hb[:],
                                            adinv_s[:, kt:kt + 1])
                nc.vector.tensor_scalar_mul(g3[:, kt, 0:C], hb[:],
                                            dinv_s[:, kt:kt + 1])
                while (next_w0 < nwin
                       and kt + 1 >= wk * next_w0 + kws[next_w0]):
                    kb0 = wk * next_w0
                    ke0 = kb0 + kws[next_w0]
                    nc.scalar.dma_start(out=b0v[:, kb0:ke0, :],
                                        in_=g3[:, kb0:ke0, :])
                    nc.gpsimd.collective_compute(
                        "AllGather", ALU.bypass, replica_groups=groups,
                        ins=[bounces[0][:][kb0 * P:ke0 * P, :].opt()],
                        outs=[tables[0][next_w0][:].opt()])
                    next_w0 += 1

            # ---------------- K propagation hops -------------------------
            gq = 0
            for hop in range(k_hops):
                last = hop == k_hops - 1
                table_wins = tables_flat[hop]
                next_w = 0
                for gi, grp in enumerate(sched):
                    mg = grp["mg"]
                    gbase = grp["pos0"] // P          # global slot base
                    npos_grp = mg * P
                    i0 = grp["pos0"] // 16
                    it = ipool.tile([P, mg * P // 16], DT.int16, tag="idx")
                    nc.sync.dma_start(
                        out=it[:],
                        in_=idx_h.ap()[:, i0:i0 + npos_grp // 16])
                    grid = gpool.tile([P, mg_max, CP], DT.float16, tag="grid")
                    for qi, (gpos0, npos, slot0, q) in enumerate(grp["gathers"]):
                        o0 = (gpos0 - grp["pos0"]) // 16
                        nc.gpsimd.dma_gather(
                            out_ap=grid[:, slot0:slot0 + npos // P, :],
                            in_ap=table_wins[q],
                            idxs_ap=it[:, o0:o0 + npos // 16],
                            num_idxs=npos,
                            num_idxs_reg=npos,
                            elem_size=CP,
                            single_packet=False,
                            queue_num=(gi + qi) % nqueues,
                        )
                        gq += 1
                    # one batched one-hot build for the whole group
                    sel = spool.tile([P, mg_max, P], DT.float16, tag="S")
                    nc.vector.tensor_tensor(
                        out=sel[:, 0:mg, :],
                        in0=iota_s[:].rearrange("p (o j) -> p o j", o=1)
                                     .broadcast_to([P, mg, P]),
                        in1=dcol_s[:, gbase:gbase + mg]
                                  .rearrange("p (m o) -> p m o", o=1)
                                  .broadcast_to([P, mg, P]),
                        op=ALU.is_equal)
                    for (kt, chunks) in grp["tiles"]:
                        psA = psbpool.tile([P, C], DT.float32, tag="agg")
                        nchunks = sum(m for _, m in chunks)
                        # self loop: previous hop's g row for this tile is
                        # still resident in g_buf (epilogue rewrites it only
                        # after these matmuls)
                        nc.tensor.matmul(
                            psA[:], lhsT=ident_s[:], rhs=g3[:, kt, 0:C],
                            start=True, stop=(nchunks == 0))
                        done = 0
                        for (slot0, m) in chunks:
                            for s in range(slot0, slot0 + m):
                                nc.tensor.matmul(
                                    psA[:], lhsT=sel[:, s, :],
                                    rhs=grid[:, s, 0:C],
                                    start=False,
                                    stop=(done == nchunks - 1))
                                done += 1
                        if not last:
                            # g <- scale2 * agg + 0.1*dinv*h0  (fp16)
                            nc.vector.scalar_tensor_tensor(
                                out=g3[:, kt, 0:C], in0=psA[:],
                                scalar=scale2_s[:, kt:kt + 1],
                                in1=h0g3[:, kt, :],
                                op0=ALU.mult, op1=ALU.add)
                        else:
                            hn = apool.tile([P, C], DT.float32, tag="hn")
                            nc.vector.scalar_tensor_tensor(
                                out=hn[:], in0=psA[:],
                                scalar=dinv09_s[:, kt:kt + 1],
                                in1=h03[:, kt, :],
                                op0=ALU.mult, op1=ALU.add)
                            mx = colpool.tile([P, 1], DT.float32, tag="mx")
                            nc.vector.reduce_max(mx[:], hn[:],
                                                 axis=mybir.AxisListType.X,
                                                 negate=True)       # -max
                            ex = apool.tile([P, C], DT.float32, tag="ex")
                            ssum = colpool.tile([P, 1], DT.float32, tag="ssum")
                            nc.scalar.activation(ex[:], hn[:], AF.Exp,
                                                 bias=mx[:, 0:1],
                                                 accum_out=ssum[:, 0:1])
                            lg = colpool.tile([P, 1], DT.float32, tag="lg")
                            nc.scalar.activation(lg[:], ssum[:], AF.Ln)
                            mpl = colpool.tile([P, 1], DT.float32, tag="mpl")
                            nc.vector.tensor_tensor(out=mpl[:], in0=lg[:],
                                                    in1=mx[:],
                                                    op=ALU.subtract)
                            res = apool.tile([P, C], DT.float32, tag="res")
                            nc.vector.tensor_scalar(
                                out=res[:], in0=hn[:],
                                scalar1=mpl[:, 0:1], scalar2=None,
                                op0=ALU.subtract)
                            ksl = slice(kt * C, (kt + 1) * C)
                            nc.sync.dma_start(out=out_h.ap()[:, ksl],
                                              in_=res[:])
                    if not last:
                        # stream this group's refreshed g rows out to the
                        # bounce buffer so each window's AllGather can fire
                        # the moment its last group's epilogue lands;
                        # ACT-issued so the Sync queue (idx loads) never
                        # stalls behind it
                        bnc = bounces[(hop + 1) % 2]
                        b3 = bnc[:].rearrange("(k p) f -> p k f", p=P)
                        k0 = grp["tiles"][0][0]
                        k1 = grp["tiles"][-1][0] + 1
                        nc.scalar.dma_start(
                            out=b3[:, k0:k1, :],
                            in_=g3[:, k0:k1, :])
                        while next_w < nwin and k1 >= wk * next_w + kws[next_w]:
                            nc.gpsimd.collective_compute(
                                "AllGather", ALU.bypass,
                                replica_groups=groups,
                                ins=[bnc[:][wk * next_w * P:
                                            (wk * next_w + kws[next_w]) * P,
                                            :].opt()],
                                outs=[tables[hop + 1][next_w][:].opt()])
                            next_w += 1

    nc.compile()
    return nc


# --------------------------------------------------------------------------
# in_maps assembly
# --------------------------------------------------------------------------

def make_in_maps(x, w1, b1, w2, b2, pre, f_in=F_IN, r=R):
    n = x.shape[0]
    t, nt = pre["t"], pre["nt"]
    fc = f_in // P

    xp = np.zeros((nt, f_in), dtype=np.float16)
    xp[:n] = np.asarray(x, dtype=np.float16)
    w1r = np.ascontiguousarray(
        np.asarray(w1, np.float16).reshape(fc, P, HID).transpose(1, 0, 2))
    b1c = np.ascontiguousarray(np.asarray(b1, np.float32).reshape(HID, 1))
    w2m = np.ascontiguousarray(np.asarray(w2, np.float16))
    b2r = np.ascontiguousarray(
        np.tile(np.asarray(b2, np.float32).reshape(1, C), (P, 1)))
    iota = np.ascontiguousarray(
        np.tile(np.arange(P, dtype=np.float16).reshape(1, P), (P, 1)))
    ident = np.eye(P, dtype=np.float16)

    in_maps = []
    for c in range(r):
        nodes = pre["shard_nodes"][c].reshape(P, t).T.reshape(-1)  # k-major
        xT_c = np.ascontiguousarray(xp[nodes].T)
        dpk = pre["dinv_pk"][c]
        in_maps.append({
            "xT": xT_c,
            "w1r": w1r, "b1c": b1c, "w2m": w2m, "b2r": b2r,
            "dinv": np.ascontiguousarray(dpk),
            "dinv09": np.ascontiguousarray(0.9 * dpk),
            "scale2": np.ascontiguousarray(0.9 * dpk * dpk),
            "adinv": np.ascontiguousarray(ALPHA * dpk),
            "idx16": pre["idx16w"][c],
            "dcol": pre["dcol"][c],
            "iota": iota,
            "ident": ident,
        })
    return in_maps


_CACHE = {}


def kernel(x, edge_index, w1, b1, w2, b2):
    from concourse.bass_utils import run_bass_kernel_spmd

    x = np.asarray(x)
    n = x.shape[0]
    pre = preprocess(np.asarray(edge_index), n)
    key = (pre["t"], pre["tot_pos"], tuple(pre["mcom"].reshape(-1)))
    if key not in _CACHE:
        _CACHE[key] = build_program(pre["t"], pre["nt"], pre["nwin"],
                                    pre["sched"], pre["tot_pos"],
                                    pre["tot_slot"])
    nc = _CACHE[key]

    in_maps = make_in_maps(x, w1, b1, w2, b2, pre)
    res = run_bass_kernel_spmd(nc, in_maps, core_ids=list(range(R)))
    outs = np.stack([res.results[c]["out"] for c in range(R)])
    flat = outs.reshape(R * pre["shard"], C)
    return np.ascontiguousarray(flat[pre["rowid"][:n]]).astype(np.float32)

